# revision 22
# baseline (speedup 1.0000x reference)
"""MixLoRA layer kernel for 8 trn2 NeuronCores.

Data-parallel over batch B=8: core c handles sample c end-to-end
(router scores, top-k, gathers, both low-rank matmuls). Expert pools /
router weights / cfs_W are replicated to every core.

Hardcoded problem shape:
  B=8, S=2048, IN=4096, OUT=4096, R=16, E=64, fp32 in/out.
"""

import numpy as np

import concourse.mybir as mybir
from concourse import bacc, bass
from concourse.bass import AP
from concourse.bass_utils import run_bass_kernel_spmd
from concourse.masks import make_identity
from concourse.tile import TileContext

F32 = mybir.dt.float32
F32R = mybir.dt.float32r
U32 = mybir.dt.uint32
I32 = mybir.dt.int32

B, S, IN, OUT, R, E = 8, 2048, 4096, 4096, 16, 64
P = 128
NEG = -1.0e30

# dtype used by the two big lora matmuls (bitcast view of the same fp32
# bits; float32r streams 1 row/cycle at N>=256 vs 4 for float32)
MM_DT = F32R
# dtype used for the x-tile PE transposes
TR_DT = F32


def _mm_cast(ap: AP) -> AP:
    if MM_DT is F32:
        return ap
    return ap.bitcast(MM_DT)


def build_nc() -> bass.Bass:
    nc = bacc.Bacc("TRN2", target_bir_lowering=False, debug=False, num_devices=B)

    x_d = nc.dram_tensor("x", [S, IN], F32, kind="ExternalInput")
    q_d = nc.dram_tensor("q", [1, IN], F32, kind="ExternalInput")
    a_pool_d = nc.dram_tensor("a_pool", [E * R, IN], F32, kind="ExternalInput")
    # B_pool (E, OUT, R) viewed as (E*16, 256*R): one row = 1/16th expert slab
    b_pool_d = nc.dram_tensor("b_pool", [E * 16, 256 * R], F32, kind="ExternalInput")
    w_ra_d = nc.dram_tensor("w_ra", [E, IN], F32, kind="ExternalInput")
    b_ra_d = nc.dram_tensor("b_ra", [1, E], F32, kind="ExternalInput")
    w_rb_d = nc.dram_tensor("w_rb", [E, IN], F32, kind="ExternalInput")
    b_rb_d = nc.dram_tensor("b_rb", [1, E], F32, kind="ExternalInput")
    # cfs_W (R, IN, E) viewed as (R, IN/128, 128, E)
    cfs_d = nc.dram_tensor("cfs", [R, IN // P, P, E], F32, kind="ExternalInput")
    out_d = nc.dram_tensor("out", [S, OUT], F32, kind="ExternalOutput")
    # scratch DRAM for on-chip partition/free reshuffles
    bounce_idx_d = nc.dram_tensor("bounce_idx", [1, P], I32)
    bounce_b_d = nc.dram_tensor("bounce_b", [P, 512], F32)

    NIT = IN // P  # 32 i-tiles of 128
    NST = S // 512  # 4 s-supertiles of 512
    NOC = OUT // 512  # 8 o-chunks of 512

    with TileContext(nc) as tc:
        with (
            tc.tile_pool(name="consts", bufs=1) as consts,
            tc.tile_pool(name="w_pool", bufs=1) as w_pool,
            tc.tile_pool(name="route_sb", bufs=1) as route_sb,
            tc.tile_pool(name="small_ps", bufs=1, space="PSUM") as small_ps,
            tc.tile_pool(name="cfs_ps_pool", bufs=1, space="PSUM") as cfs_ps_pool,
            tc.tile_pool(name="big_ps", bufs=1, space="PSUM") as big_ps,
            tc.tile_pool(name="cfs_pool", bufs=2) as cfs_pool,
            tc.tile_pool(name="big", bufs=3) as big,
            tc.tile_pool(name="gpool", bufs=1) as gpool,
            tc.tile_pool(name="x_pool", bufs=8) as x_pool,
            tc.tile_pool(name="xt_pool", bufs=3) as xt_pool,
            tc.tile_pool(name="xt_ps_pool", bufs=2, space="PSUM") as xt_ps_pool,
            tc.tile_pool(name="aft_ps_pool", bufs=1, space="PSUM") as aft_ps_pool,
            tc.tile_pool(name="delta_ps_pool", bufs=2, space="PSUM") as delta_ps_pool,
        ):
            ident = consts.tile([P, P], F32)
            make_identity(nc, ident)

            # ---------------- routing: router A ----------------
            q_sb = w_pool.tile([1, IN], F32)
            nc.sync.dma_start(out=q_sb, in_=q_d[:, :])
            w_ra_sb = w_pool.tile([E, IN], F32)
            nc.sync.dma_start(out=w_ra_sb, in_=w_ra_d[:, :])
            w_rb_sb = w_pool.tile([E, IN], F32)
            nc.sync.dma_start(out=w_rb_sb, in_=w_rb_d[:, :])
            b_ra_sb = route_sb.tile([1, E], F32)
            nc.sync.dma_start(out=b_ra_sb, in_=b_ra_d[:, :])
            b_rb_sb = route_sb.tile([1, E], F32)
            nc.sync.dma_start(out=b_rb_sb, in_=b_rb_d[:, :])

            QC = 512
            red_scr = route_sb.tile([E, QC], F32)
            ones_row = consts.tile([1, E], F32)
            nc.vector.memset(ones_row, 1.0)
            NQC = IN // QC

            def router_scores(w_sb, acc_tile, partials):
                """acc_tile [E, NQC+1]; result lands in acc_tile[:, NQC:]."""
                for c in range(NQC):
                    q64_ps = small_ps.tile([E, QC], F32, tag="sm", name="q64_ps")
                    nc.tensor.matmul(
                        out=q64_ps,
                        lhsT=ones_row,
                        rhs=q_sb[:, QC * c : QC * (c + 1)],
                        start=True,
                        stop=True,
                    )
                    nc.vector.tensor_tensor(
                        out=red_scr,
                        in0=w_sb[:, QC * c : QC * (c + 1)],
                        in1=q64_ps,
                        op=mybir.AluOpType.mult,
                    )
                    nc.vector.tensor_reduce(
                        out=acc_tile[:, c : c + 1],
                        in_=red_scr,
                        axis=mybir.AxisListType.X,
                        op=mybir.AluOpType.add,
                    )
                for lvl in range(3):
                    w = 2 ** lvl
                    for c in range(0, NQC, 2 * w):
                        nc.vector.tensor_add(
                            out=acc_tile[:, c : c + 1],
                            in0=acc_tile[:, c : c + 1],
                            in1=acc_tile[:, c + w : c + w + 1],
                        )
                nc.vector.tensor_copy(acc_tile[:, NQC : NQC + 1], acc_tile[:, 0:1])

            # gA[e] = sum_i W_rA[e,i] * q[i]  (DVE mul+reduce, [E,1] layout)
            ga_acc = route_sb.tile([E, 9], F32)
            router_scores(w_ra_sb, ga_acc, None)
            ga_col = ga_acc[:, 8:9]
            ga_ps = small_ps.tile([1, E], F32, tag="sm")
            nc.tensor.transpose(out=ga_ps, in_=ga_col, identity=ident[:E, :E])
            ga_sb = route_sb.tile([1, E], F32)
            nc.vector.tensor_add(out=ga_sb, in0=ga_ps, in1=b_ra_sb)

            def topk16(scores_sb, vals_sb, idx_sb, scratch_sb):
                """scores_sb [1,E] fp32 -> idx_sb [1,16] u32 (desc), vals in vals_sb."""
                nc.vector.max(out=vals_sb[:, 0:8], in_=scores_sb)
                nc.vector.max_index(
                    out=idx_sb[:, 0:8], in_max=vals_sb[:, 0:8], in_values=scores_sb
                )
                nc.vector.match_replace(
                    out=scratch_sb,
                    in_to_replace=vals_sb[:, 0:8],
                    in_values=scores_sb,
                    imm_value=NEG,
                )
                nc.vector.max(out=vals_sb[:, 8:16], in_=scratch_sb)
                nc.vector.max_index(
                    out=idx_sb[:, 8:16], in_max=vals_sb[:, 8:16], in_values=scratch_sb
                )

            vals_a = route_sb.tile([1, 16], F32)
            idx_a = route_sb.tile([1, 16], U32)
            tk_scr = route_sb.tile([1, E], F32)
            topk16(ga_sb, vals_a, idx_a, tk_scr)

            # row index into (E*R, IN): idx_a*16 + rank
            iota16 = route_sb.tile([1, 16], I32)
            nc.gpsimd.iota(iota16, pattern=[[1, 16]], base=0, channel_multiplier=0)
            rowf = route_sb.tile([1, 16], F32)
            idx_a_f = route_sb.tile([1, 16], F32)
            nc.vector.tensor_copy(idx_a_f, idx_a)
            nc.vector.tensor_scalar_mul(rowf, idx_a_f, 16.0)
            iota16_f = route_sb.tile([1, 16], F32)
            nc.vector.tensor_copy(iota16_f, iota16)
            nc.vector.tensor_add(out=rowf, in0=rowf, in1=iota16_f)
            row_ps = small_ps.tile([16, 1], F32, tag="sm")
            nc.tensor.transpose(out=row_ps, in_=rowf, identity=ident[:1, :1])
            row_col = route_sb.tile([16, 1], I32)
            nc.vector.tensor_copy(row_col, row_ps)

            # gather lora_A rows: [16, IN]
            lora_a = big.tile([R, IN], F32, tag="big")
            nc.gpsimd.indirect_dma_start(
                out=lora_a[:, :],
                out_offset=None,
                in_=a_pool_d[:, :],
                in_offset=bass.IndirectOffsetOnAxis(ap=row_col[:, 0:1], axis=0),
            )

            # lora_A^T tiles: [128, 16] per i-tile, packed [P, R*NIT]
            lat_ps = big_ps.tile([P, 512], F32)
            for t in range(NIT):
                nc.tensor.transpose(
                    out=lat_ps[:, 16 * t : 16 * (t + 1)],
                    in_=lora_a[:, P * t : P * (t + 1)],
                    identity=ident[:R, :R],
                )
            lora_at = consts.tile([P, R * NIT], F32)
            nc.vector.tensor_copy(lora_at, lat_ps)
            lora_at_r = consts.tile([P, R * NIT], MM_DT)
            nc.vector.tensor_copy(lora_at_r, lat_ps)

            # ---------------- cfs scores (PE, fp32) ----------------
            cfs_ps = cfs_ps_pool.tile([1, E], F32)
            for r in range(R):
                for h in range(2):
                    cfs_sb = cfs_pool.tile([P, NIT // 2, E], F32, tag="cfs")
                    nc.sync.dma_start(
                        out=cfs_sb,
                        in_=cfs_d[r, 16 * h : 16 * (h + 1)].transpose([1, 0, 2]),
                    )
                    for tt in range(NIT // 2):
                        t = 16 * h + tt
                        nc.tensor.matmul(
                            out=cfs_ps,
                            lhsT=lora_at[:, 16 * t + r : 16 * t + r + 1],
                            rhs=cfs_sb[:, tt, :],
                            start=(r == 0 and t == 0),
                            stop=(r == R - 1 and t == NIT - 1),
                        )

            # ---------------- router B ----------------
            gb_acc = route_sb.tile([E, 9], F32)
            router_scores(w_rb_sb, gb_acc, None)
            gb_col = gb_acc[:, 8:9]
            gb_ps = small_ps.tile([1, E], F32, tag="sm")
            nc.tensor.transpose(out=gb_ps, in_=gb_col, identity=ident[:E, :E])
            gb_sb = route_sb.tile([1, E], F32)
            cfs_sb_sc = route_sb.tile([1, E], F32)
            nc.vector.tensor_copy(cfs_sb_sc, cfs_ps)
            nc.vector.tensor_add(out=gb_sb, in0=gb_ps, in1=cfs_sb_sc)
            nc.vector.tensor_add(out=gb_sb, in0=gb_sb, in1=b_rb_sb)

            vals_b = route_sb.tile([1, 16], F32)
            idx_b = route_sb.tile([1, 16], U32)
            topk16(gb_sb, vals_b, idx_b, tk_scr)

            # ---- gather lora_B: lora_b[k, :] = B_pool[idx_b[k], :, k] ----
            # 1) indices ind128[p] for p = k*8+oc: idx_b[k]*8 + oc, staged
            #    through DRAM to move them onto 128 partitions.
            idx_b_f = route_sb.tile([1, 16], F32)
            nc.vector.tensor_copy(idx_b_f, idx_b)
            idx8 = route_sb.tile([1, 16], F32)
            nc.vector.tensor_scalar_mul(idx8, idx_b_f, 16.0)
            idx8_i = route_sb.tile([1, 16], I32)
            nc.vector.tensor_copy(idx8_i, idx8)
            ind2 = route_sb.tile([1, 16, 8], I32)
            iota_oc = route_sb.tile([1, 16, 8], I32)
            nc.gpsimd.iota(
                iota_oc, pattern=[[0, 16], [2, 8]], base=0, channel_multiplier=0
            )
            nc.vector.tensor_add(
                out=ind2,
                in0=iota_oc,
                in1=idx8_i.unsqueeze(2).to_broadcast([1, 16, 8]),
            )
            nc.sync.dma_start(
                out=bounce_idx_d[:, :], in_=ind2.rearrange("a b c -> a (b c)")
            )
            ind128 = route_sb.tile([P, 1], I32)
            nc.sync.dma_start(
                out=ind128,
                in_=bounce_idx_d[:, :].rearrange("a (p b) -> (a p) b", b=1),
            )
            # 2) contiguous slab gathers (two o-halves per partition row)
            # 3) mask out r != k and reduce the r dim: kmask[p, r] = (r == p//8)
            kmask = consts.tile([P, R], F32)
            nc.gpsimd.memset(kmask, 1.0)
            # v = p - 8r; keep where v >= 0
            nc.gpsimd.affine_select(
                out=kmask,
                in_=kmask,
                pattern=[[-8, R]],
                compare_op=mybir.AluOpType.is_ge,
                fill=0.0,
                base=0,
                channel_multiplier=1,
            )
            # keep where 7 - p + 8r >= 0  (i.e. p - 8r <= 7)
            nc.gpsimd.affine_select(
                out=kmask,
                in_=kmask,
                pattern=[[8, R]],
                compare_op=mybir.AluOpType.is_ge,
                fill=0.0,
                base=7,
                channel_multiplier=-1,
            )
            lorab_kb = route_sb.tile([P, 2, 256], F32)
            for h in range(2):
                gbuf = gpool.tile([P, 256, R], F32, tag="g")
                nc.gpsimd.indirect_dma_start(
                    out=gbuf.rearrange("p o r -> p (o r)"),
                    out_offset=None,
                    in_=b_pool_d[:, :],
                    in_offset=bass.IndirectOffsetOnAxis(ap=ind128[:, 0:1], axis=0),
                    element_offset=h * 256 * R,
                )
                nc.vector.tensor_tensor(
                    out=gbuf,
                    in0=gbuf,
                    in1=kmask.unsqueeze(1).to_broadcast([P, 256, R]),
                    op=mybir.AluOpType.mult,
                )
                nc.vector.tensor_reduce(
                    out=lorab_kb[:, h, :],
                    in_=gbuf,
                    axis=mybir.AxisListType.X,
                    op=mybir.AluOpType.add,
                )
            # 4) reshuffle (k*8+oc, h, 256) -> (k, 4096) through DRAM
            nc.sync.dma_start(
                out=bounce_b_d[:, :], in_=lorab_kb.rearrange("p a b -> p (a b)")
            )
            lora_b = w_pool.tile([R, OUT], MM_DT)
            nc.sync.dma_start(
                out=lora_b,
                in_=bounce_b_d[:, :].rearrange("(k c) o -> k (c o)", c=8).bitcast(
                    MM_DT
                ),
            )

            # ---------------- main pipeline ----------------
            IG = 1024  # i-columns staged per x chunk
            for st in range(NST):  # 4 supertiles of 512 s-rows
                aft_ps = aft_ps_pool.tile([R, 512], F32)
                for ig in range(IN // IG):
                    x_chunks = []
                    for sub in range(4):
                        xc = x_pool.tile([P, IG], F32, tag="x")
                        s0 = st * 512 + sub * P
                        nc.sync.dma_start(
                            out=xc, in_=x_d[s0 : s0 + P, ig * IG : (ig + 1) * IG]
                        )
                        x_chunks.append(xc)
                    for it8 in range(IG // P):
                        it = ig * (IG // P) + it8
                        xt_ps = xt_ps_pool.tile([P, 512], TR_DT)
                        for sub in range(4):
                            nc.tensor.transpose(
                                out=xt_ps[:, P * sub : P * (sub + 1)].bitcast(TR_DT),
                                in_=x_chunks[sub][:, P * it8 : P * (it8 + 1)].bitcast(
                                    TR_DT
                                ),
                                identity=ident,
                            )
                        xt_sb = xt_pool.tile([P, 512], MM_DT)
                        nc.any.tensor_copy(xt_sb, xt_ps)
                        nc.tensor.matmul(
                            out=aft_ps,
                            lhsT=lora_at_r[:, 16 * it : 16 * (it + 1)],
                            rhs=xt_sb,
                            start=(it == 0),
                            stop=(it == NIT - 1),
                        )
                aft_sb = route_sb.tile([R, 512], MM_DT, tag="aft", bufs=2)
                nc.vector.tensor_copy(aft_sb, aft_ps)

                for sub in range(4):
                    for half in range(2):
                        delta_sb = big.tile([P, OUT // 2], F32, tag="big")
                        for oc2 in range(NOC // 2):
                            oc = half * (NOC // 2) + oc2
                            delta_ps = delta_ps_pool.tile([P, 512], F32)
                            nc.tensor.matmul(
                                out=delta_ps,
                                lhsT=aft_sb[:, P * sub : P * (sub + 1)],
                                rhs=lora_b[:, 512 * oc : 512 * (oc + 1)],
                                start=True,
                                stop=True,
                            )
                            nc.any.tensor_copy(
                                delta_sb[:, 512 * oc2 : 512 * (oc2 + 1)], delta_ps
                            )
                        s0 = st * 512 + sub * P
                        o0 = half * (OUT // 2)
                        nc.sync.dma_start(
                            out=out_d[s0 : s0 + P, o0 : o0 + OUT // 2], in_=delta_sb
                        )

    nc.compile()
    return nc


def kernel(_run_kwargs=None, **inputs: np.ndarray) -> np.ndarray:
    run_kwargs = _run_kwargs or {}
    x = np.ascontiguousarray(inputs["x"], dtype=np.float32)
    q = np.ascontiguousarray(inputs["query_signal"], dtype=np.float32)
    a_pool = np.ascontiguousarray(inputs["A_pool"], dtype=np.float32).reshape(
        E * R, IN
    )
    b_pool = np.ascontiguousarray(inputs["B_pool"], dtype=np.float32).reshape(
        E * 16, 256 * R
    )
    w_ra = np.ascontiguousarray(inputs["W_rA"], dtype=np.float32)
    b_ra = np.ascontiguousarray(inputs["b_rA"], dtype=np.float32).reshape(1, E)
    w_rb = np.ascontiguousarray(inputs["W_rB"], dtype=np.float32)
    b_rb = np.ascontiguousarray(inputs["b_rB"], dtype=np.float32).reshape(1, E)
    cfs = np.ascontiguousarray(inputs["cfs_W"], dtype=np.float32).reshape(
        R, IN // P, P, E
    )

    nc = build_nc()
    in_maps = []
    for c in range(B):
        in_maps.append(
            {
                "x": np.ascontiguousarray(x[c]),
                "q": np.ascontiguousarray(q[c : c + 1]),
                "a_pool": a_pool,
                "b_pool": b_pool,
                "w_ra": w_ra,
                "b_ra": b_ra,
                "w_rb": w_rb,
                "b_rb": b_rb,
                "cfs": cfs,
            }
        )
    res = run_bass_kernel_spmd(nc, in_maps, core_ids=list(range(B)), **run_kwargs)
    if run_kwargs:
        return res
    return np.stack([r["out"] for r in res.results], axis=0)


if __name__ == "__main__":
    rng = np.random.default_rng(0)
    dummy = {
        "x": rng.standard_normal((B, S, IN), dtype=np.float32),
        "query_signal": rng.standard_normal((B, IN), dtype=np.float32),
        "A_pool": rng.standard_normal((E, R, IN), dtype=np.float32),
        "B_pool": rng.standard_normal((E, OUT, R), dtype=np.float32) * 0.02,
        "W_rA": rng.standard_normal((E, IN), dtype=np.float32) * 0.02,
        "b_rA": np.zeros(E, np.float32),
        "W_rB": rng.standard_normal((E, IN), dtype=np.float32) * 0.02,
        "b_rB": np.zeros(E, np.float32),
        "cfs_W": rng.standard_normal((R, IN, E), dtype=np.float32),
    }
    out = kernel(**dummy)
    print("out", out.shape, out.dtype, np.abs(out).max())


# revision 23
# speedup vs baseline: 3.4692x; 3.4692x over previous
"""MixLoRA layer kernel for 8 trn2 NeuronCores.

Data-parallel over batch B=8: core c handles sample c end-to-end
(router scores, top-k, gathers, both low-rank matmuls). Expert pools /
router weights / cfs_W are replicated to every core.

Hardcoded problem shape:
  B=8, S=2048, IN=4096, OUT=4096, R=16, E=64, fp32 in/out.
"""

import numpy as np

import concourse.mybir as mybir
from concourse import bacc, bass
from concourse.bass import AP
from concourse.bass_utils import run_bass_kernel_spmd
from concourse.masks import make_identity
from concourse.tile import TileContext

F32 = mybir.dt.float32
F32R = mybir.dt.float32r
U32 = mybir.dt.uint32
I32 = mybir.dt.int32

B, S, IN, OUT, R, E = 8, 2048, 4096, 4096, 16, 64
P = 128
NEG = -1.0e30

# dtype used by the two big lora matmuls (bitcast view of the same fp32
# bits; float32r streams 1 row/cycle at N>=256 vs 4 for float32)
MM_DT = F32R
# dtype used for the x-tile PE transposes
TR_DT = F32


def _mm_cast(ap: AP) -> AP:
    if MM_DT is F32:
        return ap
    return ap.bitcast(MM_DT)


def build_nc(nst=None) -> bass.Bass:
    nc = bacc.Bacc("TRN2", target_bir_lowering=False, debug=False, num_devices=B)

    x_d = nc.dram_tensor("x", [S, IN], F32, kind="ExternalInput")
    q_d = nc.dram_tensor("q", [1, IN], F32, kind="ExternalInput")
    a_pool_d = nc.dram_tensor("a_pool", [E * R, IN], F32, kind="ExternalInput")
    # B_pool (E, OUT, R) viewed as (E*16, 256*R): one row = 1/16th expert slab
    b_pool_d = nc.dram_tensor("b_pool", [E * 16, 256 * R], F32, kind="ExternalInput")
    w_ra_d = nc.dram_tensor("w_ra", [E, IN], F32, kind="ExternalInput")
    b_ra_d = nc.dram_tensor("b_ra", [1, E], F32, kind="ExternalInput")
    w_rb_d = nc.dram_tensor("w_rb", [E, IN], F32, kind="ExternalInput")
    b_rb_d = nc.dram_tensor("b_rb", [1, E], F32, kind="ExternalInput")
    # cfs_W (R, IN, E) viewed as (R, IN/128, 128, E)
    cfs_d = nc.dram_tensor("cfs", [R, IN // P, P, E], F32, kind="ExternalInput")
    out_d = nc.dram_tensor("out", [S, OUT], F32, kind="ExternalOutput")
    # scratch DRAM for on-chip partition/free reshuffles
    bounce_idx_d = nc.dram_tensor("bounce_idx", [1, P], I32)
    bounce_b_d = nc.dram_tensor("bounce_b", [P, 512], F32)

    NIT = IN // P  # 32 i-tiles of 128
    NST = (S // 512) if nst is None else nst  # s-supertiles of 512
    NOC = OUT // 512  # 8 o-chunks of 512

    with TileContext(nc) as tc:
        with (
            tc.tile_pool(name="consts", bufs=1) as consts,
            tc.tile_pool(name="w_pool", bufs=1) as w_pool,
            tc.tile_pool(name="route_sb", bufs=1) as route_sb,
            tc.tile_pool(name="small_ps", bufs=1, space="PSUM") as small_ps,
            tc.tile_pool(name="cfs_ps_pool", bufs=1, space="PSUM") as cfs_ps_pool,
            tc.tile_pool(name="big_ps", bufs=1, space="PSUM") as big_ps,
            tc.tile_pool(name="cfs_pool", bufs=2) as cfs_pool,
            tc.tile_pool(name="big", bufs=3) as big,
            tc.tile_pool(name="gpool", bufs=1) as gpool,
            tc.tile_pool(name="x_pool", bufs=8) as x_pool,
            tc.tile_pool(name="xt_pool", bufs=3) as xt_pool,
            tc.tile_pool(name="xt_ps_pool", bufs=2, space="PSUM") as xt_ps_pool,
            tc.tile_pool(name="aft_ps_pool", bufs=1, space="PSUM") as aft_ps_pool,
            tc.tile_pool(name="delta_ps_pool", bufs=2, space="PSUM") as delta_ps_pool,
        ):
            ident = consts.tile([P, P], F32)
            make_identity(nc, ident)

            # ---------------- routing: router A ----------------
            q_sb = w_pool.tile([1, IN], F32)
            nc.sync.dma_start(out=q_sb, in_=q_d[:, :])
            w_ra_sb = w_pool.tile([E, IN], F32)
            nc.sync.dma_start(out=w_ra_sb, in_=w_ra_d[:, :])
            w_rb_sb = w_pool.tile([E, IN], F32)
            nc.sync.dma_start(out=w_rb_sb, in_=w_rb_d[:, :])
            b_ra_sb = route_sb.tile([1, E], F32)
            nc.sync.dma_start(out=b_ra_sb, in_=b_ra_d[:, :])
            b_rb_sb = route_sb.tile([1, E], F32)
            nc.sync.dma_start(out=b_rb_sb, in_=b_rb_d[:, :])

            QC = 512
            red_scr = route_sb.tile([E, QC], F32)
            ones_row = consts.tile([1, E], F32)
            nc.vector.memset(ones_row, 1.0)
            NQC = IN // QC

            def router_scores(w_sb, acc_tile, partials):
                """acc_tile [E, NQC+1]; result lands in acc_tile[:, NQC:]."""
                for c in range(NQC):
                    q64_ps = small_ps.tile([E, QC], F32, tag="sm", name="q64_ps")
                    nc.tensor.matmul(
                        out=q64_ps,
                        lhsT=ones_row,
                        rhs=q_sb[:, QC * c : QC * (c + 1)],
                        start=True,
                        stop=True,
                    )
                    nc.vector.tensor_tensor(
                        out=red_scr,
                        in0=w_sb[:, QC * c : QC * (c + 1)],
                        in1=q64_ps,
                        op=mybir.AluOpType.mult,
                    )
                    nc.vector.tensor_reduce(
                        out=acc_tile[:, c : c + 1],
                        in_=red_scr,
                        axis=mybir.AxisListType.X,
                        op=mybir.AluOpType.add,
                    )
                for lvl in range(3):
                    w = 2 ** lvl
                    for c in range(0, NQC, 2 * w):
                        nc.vector.tensor_add(
                            out=acc_tile[:, c : c + 1],
                            in0=acc_tile[:, c : c + 1],
                            in1=acc_tile[:, c + w : c + w + 1],
                        )
                nc.vector.tensor_copy(acc_tile[:, NQC : NQC + 1], acc_tile[:, 0:1])

            # gA[e] = sum_i W_rA[e,i] * q[i]  (DVE mul+reduce, [E,1] layout)
            ga_acc = route_sb.tile([E, 9], F32)
            router_scores(w_ra_sb, ga_acc, None)
            ga_col = ga_acc[:, 8:9]
            ga_ps = small_ps.tile([1, E], F32, tag="sm")
            nc.tensor.transpose(out=ga_ps, in_=ga_col, identity=ident[:E, :E])
            ga_sb = route_sb.tile([1, E], F32)
            nc.vector.tensor_add(out=ga_sb, in0=ga_ps, in1=b_ra_sb)

            def topk16(scores_sb, vals_sb, idx_sb, scratch_sb):
                """scores_sb [1,E] fp32 -> idx_sb [1,16] u32 (desc), vals in vals_sb."""
                nc.vector.max(out=vals_sb[:, 0:8], in_=scores_sb)
                nc.vector.max_index(
                    out=idx_sb[:, 0:8], in_max=vals_sb[:, 0:8], in_values=scores_sb
                )
                nc.vector.match_replace(
                    out=scratch_sb,
                    in_to_replace=vals_sb[:, 0:8],
                    in_values=scores_sb,
                    imm_value=NEG,
                )
                nc.vector.max(out=vals_sb[:, 8:16], in_=scratch_sb)
                nc.vector.max_index(
                    out=idx_sb[:, 8:16], in_max=vals_sb[:, 8:16], in_values=scratch_sb
                )

            vals_a = route_sb.tile([1, 16], F32)
            idx_a = route_sb.tile([1, 16], U32)
            tk_scr = route_sb.tile([1, E], F32)
            topk16(ga_sb, vals_a, idx_a, tk_scr)

            # row index into (E*R, IN): idx_a*16 + rank
            iota16 = route_sb.tile([1, 16], I32)
            nc.gpsimd.iota(iota16, pattern=[[1, 16]], base=0, channel_multiplier=0)
            rowf = route_sb.tile([1, 16], F32)
            idx_a_f = route_sb.tile([1, 16], F32)
            nc.vector.tensor_copy(idx_a_f, idx_a)
            nc.vector.tensor_scalar_mul(rowf, idx_a_f, 16.0)
            iota16_f = route_sb.tile([1, 16], F32)
            nc.vector.tensor_copy(iota16_f, iota16)
            nc.vector.tensor_add(out=rowf, in0=rowf, in1=iota16_f)
            row_ps = small_ps.tile([16, 1], F32, tag="sm")
            nc.tensor.transpose(out=row_ps, in_=rowf, identity=ident[:1, :1])
            row_col = route_sb.tile([16, 1], I32)
            nc.vector.tensor_copy(row_col, row_ps)

            # gather lora_A rows: [16, IN]
            lora_a = big.tile([R, IN], F32, tag="big")
            nc.gpsimd.indirect_dma_start(
                out=lora_a[:, :],
                out_offset=None,
                in_=a_pool_d[:, :],
                in_offset=bass.IndirectOffsetOnAxis(ap=row_col[:, 0:1], axis=0),
            )

            # lora_A^T tiles: [128, 16] per i-tile, packed [P, R*NIT]
            lat_ps = big_ps.tile([P, 512], F32)
            for t in range(NIT):
                nc.tensor.transpose(
                    out=lat_ps[:, 16 * t : 16 * (t + 1)],
                    in_=lora_a[:, P * t : P * (t + 1)],
                    identity=ident[:R, :R],
                )
            lora_at = consts.tile([P, R * NIT], F32)
            nc.vector.tensor_copy(lora_at, lat_ps)
            lora_at_r = consts.tile([P, R * NIT], MM_DT)
            nc.vector.tensor_copy(lora_at_r, lat_ps)

            # ---------------- cfs scores (PE, fp32) ----------------
            cfs_ps = cfs_ps_pool.tile([1, E], F32)
            for r in range(R):
                for h in range(2):
                    cfs_sb = cfs_pool.tile([P, NIT // 2, E], F32, tag="cfs")
                    nc.sync.dma_start(
                        out=cfs_sb,
                        in_=cfs_d[r, 16 * h : 16 * (h + 1)].transpose([1, 0, 2]),
                    )
                    for tt in range(NIT // 2):
                        t = 16 * h + tt
                        nc.tensor.matmul(
                            out=cfs_ps,
                            lhsT=lora_at[:, 16 * t + r : 16 * t + r + 1],
                            rhs=cfs_sb[:, tt, :],
                            start=(r == 0 and t == 0),
                            stop=(r == R - 1 and t == NIT - 1),
                        )

            # ---------------- router B ----------------
            gb_acc = route_sb.tile([E, 9], F32)
            router_scores(w_rb_sb, gb_acc, None)
            gb_col = gb_acc[:, 8:9]
            gb_ps = small_ps.tile([1, E], F32, tag="sm")
            nc.tensor.transpose(out=gb_ps, in_=gb_col, identity=ident[:E, :E])
            gb_sb = route_sb.tile([1, E], F32)
            cfs_sb_sc = route_sb.tile([1, E], F32)
            nc.vector.tensor_copy(cfs_sb_sc, cfs_ps)
            nc.vector.tensor_add(out=gb_sb, in0=gb_ps, in1=cfs_sb_sc)
            nc.vector.tensor_add(out=gb_sb, in0=gb_sb, in1=b_rb_sb)

            vals_b = route_sb.tile([1, 16], F32)
            idx_b = route_sb.tile([1, 16], U32)
            topk16(gb_sb, vals_b, idx_b, tk_scr)

            # ---- gather lora_B: lora_b[k, :] = B_pool[idx_b[k], :, k] ----
            # 1) indices ind128[p] for p = k*8+oc: idx_b[k]*8 + oc, staged
            #    through DRAM to move them onto 128 partitions.
            idx_b_f = route_sb.tile([1, 16], F32)
            nc.vector.tensor_copy(idx_b_f, idx_b)
            idx8 = route_sb.tile([1, 16], F32)
            nc.vector.tensor_scalar_mul(idx8, idx_b_f, 16.0)
            idx8_i = route_sb.tile([1, 16], I32)
            nc.vector.tensor_copy(idx8_i, idx8)
            ind2 = route_sb.tile([1, 16, 8], I32)
            iota_oc = route_sb.tile([1, 16, 8], I32)
            nc.gpsimd.iota(
                iota_oc, pattern=[[0, 16], [2, 8]], base=0, channel_multiplier=0
            )
            nc.vector.tensor_add(
                out=ind2,
                in0=iota_oc,
                in1=idx8_i.unsqueeze(2).to_broadcast([1, 16, 8]),
            )
            nc.sync.dma_start(
                out=bounce_idx_d[:, :], in_=ind2.rearrange("a b c -> a (b c)")
            )
            ind128 = route_sb.tile([P, 1], I32)
            nc.sync.dma_start(
                out=ind128,
                in_=bounce_idx_d[:, :].rearrange("a (p b) -> (a p) b", b=1),
            )
            # 2) contiguous slab gathers (two o-halves per partition row)
            # 3) mask out r != k and reduce the r dim: kmask[p, r] = (r == p//8)
            kmask = consts.tile([P, R], F32)
            nc.gpsimd.memset(kmask, 1.0)
            # v = p - 8r; keep where v >= 0
            nc.gpsimd.affine_select(
                out=kmask,
                in_=kmask,
                pattern=[[-8, R]],
                compare_op=mybir.AluOpType.is_ge,
                fill=0.0,
                base=0,
                channel_multiplier=1,
            )
            # keep where 7 - p + 8r >= 0  (i.e. p - 8r <= 7)
            nc.gpsimd.affine_select(
                out=kmask,
                in_=kmask,
                pattern=[[8, R]],
                compare_op=mybir.AluOpType.is_ge,
                fill=0.0,
                base=7,
                channel_multiplier=-1,
            )
            lorab_kb = route_sb.tile([P, 2, 256], F32)
            for h in range(2):
                gbuf = gpool.tile([P, 256, R], F32, tag="g")
                nc.gpsimd.indirect_dma_start(
                    out=gbuf.rearrange("p o r -> p (o r)"),
                    out_offset=None,
                    in_=b_pool_d[:, :],
                    in_offset=bass.IndirectOffsetOnAxis(ap=ind128[:, 0:1], axis=0),
                    element_offset=h * 256 * R,
                )
                nc.vector.tensor_tensor(
                    out=gbuf,
                    in0=gbuf,
                    in1=kmask.unsqueeze(1).to_broadcast([P, 256, R]),
                    op=mybir.AluOpType.mult,
                )
                nc.vector.tensor_reduce(
                    out=lorab_kb[:, h, :],
                    in_=gbuf,
                    axis=mybir.AxisListType.X,
                    op=mybir.AluOpType.add,
                )
            # 4) reshuffle (k*8+oc, h, 256) -> (k, 4096) through DRAM
            nc.sync.dma_start(
                out=bounce_b_d[:, :], in_=lorab_kb.rearrange("p a b -> p (a b)")
            )
            lora_b = w_pool.tile([R, OUT], MM_DT)
            nc.sync.dma_start(
                out=lora_b,
                in_=bounce_b_d[:, :].rearrange("(k c) o -> k (c o)", c=8).bitcast(
                    MM_DT
                ),
            )

            # ---------------- main pipeline ----------------
            IG = 1024  # i-columns staged per x chunk
            for st in range(NST):  # 4 supertiles of 512 s-rows
                aft_ps = aft_ps_pool.tile([R, 512], F32)
                for ig in range(IN // IG):
                    x_chunks = []
                    for sub in range(4):
                        xc = x_pool.tile([P, IG], F32, tag="x")
                        s0 = st * 512 + sub * P
                        nc.sync.dma_start(
                            out=xc, in_=x_d[s0 : s0 + P, ig * IG : (ig + 1) * IG]
                        )
                        x_chunks.append(xc)
                    for it8 in range(IG // P):
                        it = ig * (IG // P) + it8
                        xt_ps = xt_ps_pool.tile([P, 512], TR_DT)
                        for sub in range(4):
                            nc.tensor.transpose(
                                out=xt_ps[:, P * sub : P * (sub + 1)].bitcast(TR_DT),
                                in_=x_chunks[sub][:, P * it8 : P * (it8 + 1)].bitcast(
                                    TR_DT
                                ),
                                identity=ident,
                            )
                        xt_sb = xt_pool.tile([P, 512], MM_DT)
                        nc.any.tensor_copy(xt_sb, xt_ps)
                        nc.tensor.matmul(
                            out=aft_ps,
                            lhsT=lora_at_r[:, 16 * it : 16 * (it + 1)],
                            rhs=xt_sb,
                            start=(it == 0),
                            stop=(it == NIT - 1),
                        )
                aft_sb = route_sb.tile([R, 512], MM_DT, tag="aft", bufs=2)
                nc.vector.tensor_copy(aft_sb, aft_ps)

                for sub in range(4):
                    for half in range(2):
                        delta_sb = big.tile([P, OUT // 2], F32, tag="big")
                        for oc2 in range(NOC // 2):
                            oc = half * (NOC // 2) + oc2
                            delta_ps = delta_ps_pool.tile([P, 512], F32)
                            nc.tensor.matmul(
                                out=delta_ps,
                                lhsT=aft_sb[:, P * sub : P * (sub + 1)],
                                rhs=lora_b[:, 512 * oc : 512 * (oc + 1)],
                                start=True,
                                stop=True,
                            )
                            nc.any.tensor_copy(
                                delta_sb[:, 512 * oc2 : 512 * (oc2 + 1)], delta_ps
                            )
                        s0 = st * 512 + sub * P
                        o0 = half * (OUT // 2)
                        nc.sync.dma_start(
                            out=out_d[s0 : s0 + P, o0 : o0 + OUT // 2], in_=delta_sb
                        )

    nc.compile()
    return nc


def kernel(_run_kwargs=None, **inputs: np.ndarray) -> np.ndarray:
    run_kwargs = _run_kwargs or {}
    x = np.ascontiguousarray(inputs["x"], dtype=np.float32)
    q = np.ascontiguousarray(inputs["query_signal"], dtype=np.float32)
    a_pool = np.ascontiguousarray(inputs["A_pool"], dtype=np.float32).reshape(
        E * R, IN
    )
    b_pool = np.ascontiguousarray(inputs["B_pool"], dtype=np.float32).reshape(
        E * 16, 256 * R
    )
    w_ra = np.ascontiguousarray(inputs["W_rA"], dtype=np.float32)
    b_ra = np.ascontiguousarray(inputs["b_rA"], dtype=np.float32).reshape(1, E)
    w_rb = np.ascontiguousarray(inputs["W_rB"], dtype=np.float32)
    b_rb = np.ascontiguousarray(inputs["b_rB"], dtype=np.float32).reshape(1, E)
    cfs = np.ascontiguousarray(inputs["cfs_W"], dtype=np.float32).reshape(
        R, IN // P, P, E
    )

    nc = build_nc()
    in_maps = []
    for c in range(B):
        in_maps.append(
            {
                "x": np.ascontiguousarray(x[c]),
                "q": np.ascontiguousarray(q[c : c + 1]),
                "a_pool": a_pool,
                "b_pool": b_pool,
                "w_ra": w_ra,
                "b_ra": b_ra,
                "w_rb": w_rb,
                "b_rb": b_rb,
                "cfs": cfs,
            }
        )
    res = run_bass_kernel_spmd(nc, in_maps, core_ids=list(range(B)), **run_kwargs)
    if run_kwargs:
        return res
    return np.stack([r["out"] for r in res.results], axis=0)


if __name__ == "__main__":
    rng = np.random.default_rng(0)
    dummy = {
        "x": rng.standard_normal((B, S, IN), dtype=np.float32),
        "query_signal": rng.standard_normal((B, IN), dtype=np.float32),
        "A_pool": rng.standard_normal((E, R, IN), dtype=np.float32),
        "B_pool": rng.standard_normal((E, OUT, R), dtype=np.float32) * 0.02,
        "W_rA": rng.standard_normal((E, IN), dtype=np.float32) * 0.02,
        "b_rA": np.zeros(E, np.float32),
        "W_rB": rng.standard_normal((E, IN), dtype=np.float32) * 0.02,
        "b_rB": np.zeros(E, np.float32),
        "cfs_W": rng.standard_normal((R, IN, E), dtype=np.float32),
    }
    out = kernel(**dummy)
    print("out", out.shape, out.dtype, np.abs(out).max())
